# revision 27
# baseline (speedup 1.0000x reference)
"""GQA causal attention (LLaMA rope) on 8 TRN2 NeuronCores.

Sharding: 8 cores = 2 batches x 4 kv-head-groups. Each core owns one batch
element, 2 kv heads and their 8 query heads (q head h uses kv head h % 8).
Host prep transposes/casts inputs (x^T bf16, pair-permuted w_qkv columns so
RoPE becomes contiguous-partition-block ops, slab-concatenated weight
layouts for single-DMA loads, tiled cos/sin tables, doubled causal triangle
mask, identity for PE transposes). On-device per core:
  1. qkv^T = w_all^T @ x^T via bf16 matmuls (transposed layout: head-dim on
     partitions) + RoPE on DVE + partition re-layout, interleaved per chunk
     so DVE stays ahead. v is also computed transposed (N=512 moving, 4x
     fewer PE instructions than the natural layout) then PE-transposed back
     per 128-block into the [v | ones] PV stationary tiles.
  2. Flash-style attention, S^T layout ([k-rows, q-cols]) over 512-wide q
     windows. The two heads of a pair share one kv head, so their score
     tiles live in one [128, 1024] PSUM pair tile: QK^T per head (K=32
     row-packed x1/x2 pair, PE-array row groups 0/64 run concurrently),
     ONE exp instruction per k-block covering both halves (3D AP), causal
     mask multiply on both diagonal blocks in one op, and ONE PV matmul
     per k-block accumulating both heads ([v | ones] stationary, row 64 =
     softmax sums). Output projection of the previous window drains into
     the current window's k-block slots to fill PE bubbles.
  3. Per window: sum rows gathered by ScalarE into one [8, W] tile, ONE
     DVE reciprocal, bf16 broadcast (GpSimd) and 4x-mode normalize
     multiplies, then out_partial = attn^T.T @ w_o_shard, written bf16.
Host gather: out[b] = sum of the 4 head-group partials + b_o.
"""

import os
import sys

sys.path.insert(0, "/opt/trn_rl_repo")

from contextlib import ExitStack

import numpy as np
import ml_dtypes

import concourse.bass as bass
import concourse.mybir as mybir
from concourse import bacc
import concourse.tile as tile
from concourse.bass import ds, ts

dt = mybir.dt
F32 = dt.float32
BF16 = dt.bfloat16
AF = mybir.ActivationFunctionType

D = 2048          # d_model
HD = 64           # head dim
NQ = 8            # local q heads per core
NKV = 2           # local kv heads per core
QR = NQ * HD      # 512 local q rows
ROPE_THETA = 10000.0
BF = ml_dtypes.bfloat16


PAIR_PV = os.environ.get("V4_PAIR_PV", "0") == "1"     # one 3D-AP PV matmul (fails walrus ISA check)
PAIR_EXP = os.environ.get("V4_PAIR_EXP", "1") == "1"   # one 3D-AP exp per kb
PAIR_MASK = os.environ.get("V4_PAIR_MASK", "1") == "1" # one 3D-AP mask per kb
VT_TRANS = os.environ.get("V4_VT_TRANS", "1") == "1"   # transposed v proj


def make_windows(S):
    ws = []
    qb = 0
    while qb < S:
        w = min(512, S - qb)
        ws.append((qb, w))
        qb += w
    return ws


def build_nc(S):
    """Build the single-core Bass/Tile graph for sequence length S."""
    SC = min(512, S)          # seq chunk for phase-1 matmuls
    NSLAB = D // 128          # 16 contraction slabs
    NCH = S // SC             # phase-1 chunks
    NST = S // 128            # seq 128-blocks

    nc = bacc.Bacc("TRN2", target_bir_lowering=False, debug=False)

    xt = nc.dram_tensor("xt", [D, S], BF16, kind="ExternalInput").ap()
    wall = nc.dram_tensor("wall", [128, NSLAB * 640], BF16, kind="ExternalInput").ap()
    wv = nc.dram_tensor("wv", [128, NSLAB * NKV * HD], BF16, kind="ExternalInput").ap()
    wo = nc.dram_tensor("wo", [128, 4 * D], BF16, kind="ExternalInput").ap()
    cs = nc.dram_tensor("cs", [128, S], BF16, kind="ExternalInput").ap()
    sn = nc.dram_tensor("sn", [128, S], BF16, kind="ExternalInput").ap()
    tri2 = nc.dram_tensor("tri2", [128, 256], BF16, kind="ExternalInput").ap()
    idm = nc.dram_tensor("idm", [128, 128], BF16, kind="ExternalInput").ap()
    out = nc.dram_tensor("out", [S, D], BF16, kind="ExternalOutput").ap()

    with tile.TileContext(nc) as tc, ExitStack() as ctx:
        const = ctx.enter_context(tc.tile_pool(name="const", bufs=1))
        tmp = ctx.enter_context(tc.tile_pool(name="tmp", bufs=2))
        ptp = ctx.enter_context(tc.tile_pool(name="ptp", bufs=3))
        recp = ctx.enter_context(tc.tile_pool(name="recp", bufs=2))
        mmp = ctx.enter_context(tc.tile_pool(name="mmp", bufs=2, space="PSUM"))
        pvp = ctx.enter_context(tc.tile_pool(name="pvp", bufs=2, space="PSUM"))

        # ---- persistent SBUF tensors -------------------------------------
        xt_sb = const.tile([128, NSLAB * S], BF16, tag="xt")
        wall_sb = const.tile([128, NSLAB * 640], BF16, tag="wall")
        wv_sb = const.tile([128, NSLAB * NKV * HD], BF16, tag="wv")
        cs_sb = const.tile([128, S], BF16, tag="cs")
        sn_sb = const.tile([128, S], BF16, tag="sn")
        tri2_sb = const.tile([128, 256], BF16, tag="tri2")
        id_sb = const.tile([128, 128], BF16, tag="idm")
        # post-rope q/k in transposed layout (bf16)
        qx1_sb = [const.tile([128, S], BF16, tag=f"qx1{i}", name=f"qx1{i}") for i in range(2)]
        qx2_sb = [const.tile([128, S], BF16, tag=f"qx2{i}", name=f"qx2{i}") for i in range(2)]
        kx_sb = const.tile([128, S], BF16, tag="kx")  # rows 0:64 x1, 64:128 x2
        # v^T staging + v natural [v_kv0 | 1 | v_kv1 | 1] PV stationaries
        vt_sb = const.tile([128, S], BF16, tag="vt")
        vones = [const.tile([128, 2 * (HD + 1)], BF16, tag=f"vo{i}", name=f"vo{i}")
                 for i in range(NST)]
        # attention output, transposed (head-dim rows): 4 tiles of 128 rows
        att_sb = [const.tile([128, S], BF16, tag=f"at{i}", name=f"at{i}") for i in range(4)]
        # partition-aligned attention inputs (own space so relayout can run
        # ahead of the xt readers): per head pair [x1(32)|x2(32)] strips
        qh_sb = [const.tile([128, S], BF16, tag=f"qh{i}", name=f"qh{i}")
                 for i in range(4)]
        kt2_sb = [const.tile([128, S], BF16, tag=f"kt2{i}", name=f"kt2{i}")
                  for i in range(NKV)]

        # ---- input DMAs: wall chunks interleaved with the xt slabs that
        # consume them so phase-1 starts within a few us
        def dma_in():
            for c in range(4):
                nc.sync.dma_start(wall_sb[:, ds(c * 4 * 640, 4 * 640)],
                                  wall[:, ds(c * 4 * 640, 4 * 640)])
                nc.sync.dma_start(
                    xt_sb[:, ds(4 * c * S, 4 * S)],
                    xt[ds(512 * c, 512), :].rearrange("(i p) s -> p i s",
                                                      p=128))
                if c == 0:
                    nc.sync.dma_start(cs_sb[:, :], cs[:, :])
                    nc.sync.dma_start(sn_sb[:, :], sn[:, :])
                elif c == 1:
                    nc.sync.dma_start(wv_sb[:, :], wv[:, :])
                elif c == 2:
                    nc.sync.dma_start(id_sb[:, :], idm[:, :])
                    nc.sync.dma_start(tri2_sb[:, :], tri2[:, :])
        dma_in()

        # ---- phase 1: qkv^T projection + RoPE + relayout -----------------
        def rope(x1, x2, d1, d2, c, s, P):
            """d1 = x1*c - x2*s ; d2 = x1*s + x2*c (writes bf16).

            PSUM f32 inputs are cast to bf16 once so the six elementwise ops
            run in the DVE 4x (2-byte packed SBUF) mode."""
            xb1 = tmp.tile([P, SC], BF16, tag="ra")
            xb2 = tmp.tile([P, SC], BF16, tag="rb")
            nc.vector.tensor_copy(xb1[:, :], x1)
            nc.vector.tensor_copy(xb2[:, :], x2)
            a = tmp.tile([P, SC], BF16, tag="rc")
            b = tmp.tile([P, SC], BF16, tag="rc")
            nc.vector.tensor_mul(a[:, :], xb1[:, :], c)
            nc.vector.tensor_mul(b[:, :], xb2[:, :], s)
            nc.vector.tensor_sub(d1, a[:, :], b[:, :])
            a2 = tmp.tile([P, SC], BF16, tag="rb")
            b2 = tmp.tile([P, SC], BF16, tag="ra")
            nc.vector.tensor_mul(a2[:, :], xb1[:, :], s)
            nc.vector.tensor_mul(b2[:, :], xb2[:, :], c)
            nc.vector.tensor_add(d2, a2[:, :], b2[:, :])

        def proj_chunk(sc_i):
            scs = ds(sc_i * SC, SC)
            for t in range(2):  # q tile pairs: x1 rows (t), x2 rows (2+t)
                ps1 = mmp.tile([128, SC], F32, tag="mm")
                for kk in range(NSLAB):
                    nc.tensor.matmul(ps1[:, :], wall_sb[:, ds(640 * kk + 128 * t, 128)],
                                     xt_sb[:, ds(S * kk + sc_i * SC, SC)],
                                     start=(kk == 0), stop=(kk == NSLAB - 1))
                ps2 = mmp.tile([128, SC], F32, tag="mm")
                for kk in range(NSLAB):
                    nc.tensor.matmul(ps2[:, :], wall_sb[:, ds(640 * kk + 256 + 128 * t, 128)],
                                     xt_sb[:, ds(S * kk + sc_i * SC, SC)],
                                     start=(kk == 0), stop=(kk == NSLAB - 1))
                rope(ps1[:, :], ps2[:, :],
                     qx1_sb[t][:, scs], qx2_sb[t][:, scs],
                     cs_sb[:, scs], sn_sb[:, scs], 128)
            # k tile: psum rows 0:64 = kx1, 64:128 = kx2
            psk = mmp.tile([128, SC], F32, tag="mm")
            for kk in range(NSLAB):
                nc.tensor.matmul(psk[:, :], wall_sb[:, ds(640 * kk + 512, 128)],
                                 xt_sb[:, ds(S * kk + sc_i * SC, SC)],
                                 start=(kk == 0), stop=(kk == NSLAB - 1))
            rope(psk[0:64, :], psk[64:128, :],
                 kx_sb[0:64, scs], kx_sb[64:128, scs],
                 cs_sb[0:64, scs], sn_sb[0:64, scs], 64)
            # v^T projection for this chunk (N=512 moving like q/k)
            if VT_TRANS:
                psv = mmp.tile([128, SC], F32, tag="mm")
                for kk in range(NSLAB):
                    nc.tensor.matmul(psv[:, :],
                                     wv_sb[:, ds(NKV * HD * kk, NKV * HD)],
                                     xt_sb[:, ds(S * kk + sc_i * SC, SC)],
                                     start=(kk == 0), stop=(kk == NSLAB - 1))
                nc.vector.tensor_copy(vt_sb[:, scs], psv[:, :])
            # partition-aligned re-layout for attention: interleave per head
            # [x1(32) | x2(32)] strips, two heads per tile; replicate each kv
            # head into both strips.
            for qt in range(4):
                for u in range(2):  # head 2*qt + u
                    l = 2 * qt + u
                    t, r0 = l // 4, 32 * (l % 4)
                    nc.vector.tensor_copy(qh_sb[qt][64 * u:64 * u + 32, scs],
                                          qx1_sb[t][r0:r0 + 32, scs])
                    nc.vector.tensor_copy(
                        qh_sb[qt][64 * u + 32:64 * u + 64, scs],
                        qx2_sb[t][r0:r0 + 32, scs])
            for j in range(NKV):
                for u in range(2):
                    nc.vector.tensor_copy(kt2_sb[j][64 * u:64 * u + 32, scs],
                                          kx_sb[32 * j:32 * j + 32, scs])
                    nc.vector.tensor_copy(
                        kt2_sb[j][64 * u + 32:64 * u + 64, scs],
                        kx_sb[64 + 32 * j:64 + 32 * j + 32, scs])

        def v_transpose(st_lo, st_hi):
            for st_i in range(st_lo, st_hi):
                if VT_TRANS:
                    tr = mmp.tile([128, 128], BF16, tag="mm", name=f"tr{st_i}")
                    nc.tensor.transpose(tr[:, :], vt_sb[:, ts(st_i, 128)],
                                        id_sb[:, :])
                else:
                    tr = mmp.tile([128, 128], F32, tag="mm", name=f"tr{st_i}")
                    for kk in range(NSLAB):
                        nc.tensor.matmul(tr[:, :],
                                         xt_sb[:, ds(S * kk + st_i * 128, 128)],
                                         wv_sb[:, ds(NKV * HD * kk, NKV * HD)],
                                         start=(kk == 0),
                                         stop=(kk == NSLAB - 1))
                vt = vones[st_i]
                nc.vector.tensor_copy(vt[:, 0:HD], tr[:, 0:HD])
                nc.vector.tensor_copy(vt[:, HD + 1:2 * HD + 1], tr[:, HD:2 * HD])
                nc.vector.memset(vt[:, HD:HD + 1], 1.0)
                nc.vector.memset(vt[:, 2 * HD + 1:2 * HD + 2], 1.0)

        half = (NCH + 1) // 2
        for sc_i in range(half):
            proj_chunk(sc_i)
        v_transpose(0, half * SC // 128)
        for sc_i in range(half, NCH):
            proj_chunk(sc_i)
        v_transpose(half * SC // 128, NST)

        # w_o loads reuse the xt slab slots (all xt reads are done by now)
        wo_sb = const.tile([128, 4 * D], BF16, tag="xt", name="wos")
        nc.sync.dma_start(wo_sb[:, :], wo[:, :])

        # ---- phases 2+3: attention + fused output projection -------------
        # Window-outer / head-pair-inner. PV lags QK/exp by one k-block so
        # the PE never waits on ScalarE. Output projection of window w-1 is
        # drained into window w's kb slots to fill PE bubbles while ACT
        # runs exp; remaining groups flush at the end.
        pending = []

        def queue_outproj(qb, W):
            for st_i in range(qb // 128, (qb + W) // 128):
                for nt in range(D // 512):
                    def g(st_i=st_i, nt=nt):
                        po = mmp.tile([128, 512], F32, tag="mm",
                                      name=f"po{st_i}_{nt}")
                        for kk in range(4):
                            nc.tensor.matmul(po[:, :],
                                             att_sb[kk][:, ts(st_i, 128)],
                                             wo_sb[:, ds(D * kk + 512 * nt, 512)],
                                             start=(kk == 0), stop=(kk == 3))
                        ot = tmp.tile([128, 512], BF16, tag="ot",
                                      name=f"ot{st_i}_{nt}")
                        nc.vector.tensor_copy(ot[:, :], po[:, :])
                        nc.sync.dma_start(
                            out[ds(st_i * 128, 128), ts(nt, 512)], ot[:, :])
                    pending.append(g)

        def pair3(t, off, ln, plo=0, phi=128):
            """[plo:phi, (2, ln)] AP over both halves of a [*, 2*W] tile."""
            return t[plo:phi, :].rearrange("p (u n) -> p u n", u=2)[
                :, :, ds(off, ln)]

        for wi, (qb, W) in enumerate(make_windows(S)):
            nkb = (qb + W) // 128
            last_w = qb + W >= S
            # hold 4 outproj groups for the window-end normalize chain
            budget = len(pending) - (0 if last_w else 4)
            for hp in range(4):
                kvl = hp // 2  # both heads of the pair share this kv head
                pvt = pvp.tile([128, 2 * W], F32, tag="pv",
                               name=f"pv{hp}_{qb}")
                lag = None  # (kb, pt, o)
                for kb in range(nkb):
                    kpos = kb * 128
                    o = max(kpos - qb, 0)
                    stp = mmp.tile([128, 2 * W], F32, tag="mm",
                                   name=f"st{hp}_{qb}_{kb}")
                    pt = ptp.tile([128, 2 * W], BF16, tag="pt",
                                  name=f"pt{hp}_{qb}_{kb}")
                    # QK^T: one K=64 matmul per head; pair lands in distinct
                    # row groups (base 0/64) -> concurrent on PE. Each half
                    # of the pair tile is one PSUM bank (W=512).
                    for u in range(2):
                        b0 = 64 * u
                        nc.tensor.matmul(
                            stp[:, ds(u * W + o, W - o)],
                            kt2_sb[kvl][b0:b0 + 64, ds(kpos, 128)],
                            qh_sb[hp][b0:b0 + 64, ds(qb + o, W - o)],
                            start=True, stop=True)
                    # exp(S/8) -> bf16 P^T, both heads in one instruction
                    if PAIR_EXP:
                        nc.scalar.activation(pair3(pt, o, W - o),
                                             pair3(stp, o, W - o),
                                             AF.Exp, scale=0.125)
                    else:
                        for u in range(2):
                            nc.scalar.activation(
                                pt[:, ds(u * W + o, W - o)],
                                stp[:, ds(u * W + o, W - o)],
                                AF.Exp, scale=0.125)
                    # causal triangle mask on both diagonal blocks
                    if kpos >= qb:
                        if PAIR_MASK:
                            nc.vector.tensor_mul(pair3(pt, o, 128),
                                                 pair3(pt, o, 128),
                                                 tri2_sb[:, :])
                        else:
                            for u in range(2):
                                nc.vector.tensor_mul(
                                    pt[:, ds(u * W + o, 128)],
                                    pt[:, ds(u * W + o, 128)],
                                    tri2_sb[:, 0:128])

                    def pv_mm(lkb, lpt, lo):
                        if PAIR_PV:
                            nc.tensor.matmul(
                                pair3(pvt, lo, W - lo, 0, 65),
                                vones[lkb][:, ds(kvl * (HD + 1), HD + 1)],
                                pair3(lpt, lo, W - lo),
                                start=(lkb == 0), stop=(lkb == nkb - 1))
                        else:
                            for u in range(2):
                                nc.tensor.matmul(
                                    pvt[0:65, ds(u * W + lo, W - lo)],
                                    vones[lkb][:, ds(kvl * (HD + 1), HD + 1)],
                                    lpt[:, ds(u * W + lo, W - lo)],
                                    start=(lkb == 0), stop=(lkb == nkb - 1))

                    if lag is not None:
                        pv_mm(*lag)
                    # outproj of window w-1 depends on its normalize chain
                    # (recip on DVE); drain only after ~2 head-pairs of PE
                    # work so a pop never blocks the in-order PE queue
                    if hp >= 2 and pending and budget > 0:
                        pending.pop(0)()
                        budget -= 1
                    lag = (kb, pt, o)
                pv_mm(*lag)
                # offload pv psum -> SBUF bf16 (frees the psum slot); pair
                # sum rows gathered into 32-aligned partition slots of the
                # shared per-window tile (engines require 32-aligned bases)
                if hp == 0:
                    sums8 = recp.tile([128, 2 * W], F32, tag="sums", bufs=1,
                                      name=f"sums{qb}")
                    pvsbs = []
                pvsb = recp.tile([HD, 2 * W], BF16, tag="pvsb", bufs=4,
                                 name=f"pvsb{hp}_{qb}")
                if hp % 2 == 0:
                    nc.vector.tensor_copy(pvsb[:, :], pvt[0:HD, :])
                else:
                    nc.scalar.activation(pvsb[:, :], pvt[0:HD, :], AF.Copy)
                nc.scalar.activation(sums8[32 * hp:32 * hp + 1, :],
                                     pvt[64:65, :], AF.Copy)
                pvsbs.append(pvsb)
            # ---- per-window normalize: one reciprocal for all 8 heads ----
            rec8 = recp.tile([128, 2 * W], F32, tag="rec8", bufs=1,
                             name=f"rec8{qb}")
            nc.vector.reciprocal(rec8[:, :], sums8[:, :])
            for hp in range(4):
                for u in range(2):
                    h = 2 * hp + u
                    # partition_broadcast needs a partition-0 zero-offset
                    # source: stage the head's rec row (f32 -> bf16 cast)
                    recs = recp.tile([1, W], BF16, tag="recs",
                                     name=f"recs{h}_{qb}")
                    nc.vector.tensor_copy(
                        recs[:, :], rec8[32 * hp:32 * hp + 1, ds(u * W, W)])
                    bcs = recp.tile([HD, W], BF16, tag="bcs",
                                    name=f"bcs{h}_{qb}")
                    nc.gpsimd.partition_broadcast(bcs[:, :], recs[:, :],
                                                  channels=HD)
                    att_dst = att_sb[h // 2][64 * (h % 2):64 * (h % 2) + 64,
                                             ds(qb, W)]
                    nc.vector.tensor_mul(att_dst, pvsbs[hp][:, ds(u * W, W)],
                                         bcs[:, :])
                if pending:
                    pending.pop(0)()
            queue_outproj(qb, W)
        while pending:
            pending.pop(0)()
    nc.compile()
    return nc


# ---------------------------------------------------------------------------
# host-side prep / gather
# ---------------------------------------------------------------------------

def _slabcat(w, slab_rows=128):
    """[R, C] -> [slab_rows, (R//slab_rows)*C] slab-concatenated bf16."""
    r, c = w.shape
    n = r // slab_rows
    return np.ascontiguousarray(
        w.reshape(n, slab_rows, c).transpose(1, 0, 2).reshape(slab_rows, n * c)
    ).astype(BF)


def _core_inputs(x, w_qkv, w_o, S):
    """Per-core input dicts. Core 4*b+g: batch b, kv heads {2g, 2g+1}."""
    E = np.arange(0, HD, 2)
    O = np.arange(1, HD, 2)
    inv_freq = 1.0 / (ROPE_THETA ** (np.arange(0, HD, 2, dtype=np.float64) / HD))
    ang = np.arange(S, dtype=np.float64)[None, :] * inv_freq[:, None]  # [32,S]
    cs = np.tile(np.cos(ang), (4, 1)).astype(BF)
    sn = np.tile(np.sin(ang), (4, 1)).astype(BF)
    r = np.arange(128)
    tri = (r[:, None] <= r[None, :]).astype(BF)  # allow k<=q
    tri2 = np.ascontiguousarray(np.concatenate([tri, tri], axis=1))
    idm = np.eye(128, dtype=np.float32).astype(BF)

    maps = []
    for b in range(2):
        for g in range(4):
            qh = [2 * g, 2 * g + 8, 2 * g + 16, 2 * g + 24,
                  2 * g + 1, 2 * g + 9, 2 * g + 17, 2 * g + 25]
            kvh = [2 * g, 2 * g + 1]
            qx1_cols = np.concatenate([64 * h + E for h in qh])
            qx2_cols = np.concatenate([64 * h + O for h in qh])
            kx1_cols = np.concatenate([D + 64 * j + E for j in kvh])
            kx2_cols = np.concatenate([D + 64 * j + O for j in kvh])
            wall_cols = np.concatenate([qx1_cols, qx2_cols, kx1_cols, kx2_cols])
            wv_cols = np.concatenate(
                [D + NKV * 4 * HD + 64 * j + np.arange(HD) for j in kvh])
            wo_rows = np.concatenate([64 * h + np.arange(HD) for h in qh])
            maps.append({
                "xt": np.ascontiguousarray(x[b].T).astype(BF),
                "wall": _slabcat(w_qkv[:, wall_cols]),
                "wv": _slabcat(w_qkv[:, wv_cols]),
                "wo": _slabcat(w_o[wo_rows, :]),
                "cs": cs, "sn": sn, "tri2": tri2, "idm": idm,
            })
    return maps


def _install_axon_ntff_hook():
    """Provide antenv.axon_hooks via ctypes on libaxon_pjrt.so if missing."""
    try:
        from antenv.axon_hooks import get_axon_ntff_profile_hook  # noqa: F401
        return
    except ImportError:
        pass
    import contextlib
    import ctypes
    import types

    so_path = "/opt/axon/libaxon_pjrt.so"
    hook = None
    if os.path.exists(so_path):
        lib = ctypes.CDLL(so_path)
        if hasattr(lib, "axon_start_nrt_profile"):
            lib.axon_start_nrt_profile.argtypes = [
                ctypes.POINTER(ctypes.c_int64), ctypes.c_size_t]
            lib.axon_start_nrt_profile.restype = ctypes.c_int64
            lib.axon_stop_nrt_profile.argtypes = [ctypes.c_char_p]
            lib.axon_stop_nrt_profile.restype = ctypes.c_int64

            @contextlib.contextmanager
            def _hook(output_dir, device_ids):
                import jax
                jax.devices()
                if device_ids:
                    ids = (ctypes.c_int64 * len(device_ids))(*device_ids)
                    rc = lib.axon_start_nrt_profile(ids, len(device_ids))
                else:
                    rc = lib.axon_start_nrt_profile(None, 0)
                if rc != 0:
                    raise RuntimeError(f"axon_start_nrt_profile rc={rc}")
                try:
                    yield
                finally:
                    n = lib.axon_stop_nrt_profile(str(output_dir).encode())
                    print(f"ntff profile: {n} file(s) -> {output_dir}")

            hook = _hook

    import antenv
    mod = types.ModuleType("antenv.axon_hooks")
    state = {"hook": hook}
    mod.get_axon_ntff_profile_hook = lambda: state["hook"]
    mod.set_axon_ntff_profile_hook = lambda h: state.__setitem__("hook", h)
    sys.modules["antenv.axon_hooks"] = mod
    antenv.axon_hooks = mod


_NC_CACHE = {}


def kernel(x, w_qkv, b_qkv, w_o, b_o):
    x = np.asarray(x, dtype=np.float32)
    w_qkv = np.asarray(w_qkv, dtype=np.float32)
    w_o = np.asarray(w_o, dtype=np.float32)
    b_o = np.asarray(b_o, dtype=np.float32)
    S = x.shape[1]

    from concourse.bass_utils import run_bass_kernel_spmd

    if S not in _NC_CACHE:
        _NC_CACHE[S] = build_nc(S)
    nc = _NC_CACHE[S]

    in_maps = _core_inputs(x, w_qkv, w_o, S)
    trace = os.environ.get("BASS_KERNEL_TRACE", "0") == "1"
    tmpdir = None
    if trace:
        _install_axon_ntff_hook()
        import concourse.bass_utils as bu
        bu.upload_artifacts = lambda d: f"local://{d}"
        tmpdir = os.environ.get("BASS_KERNEL_TRACE_DIR") or None
        if tmpdir:
            import uuid
            tmpdir = os.path.join(tmpdir, uuid.uuid4().hex[:8])
            os.makedirs(tmpdir, exist_ok=True)
        kernel.last_trace_dir = tmpdir
    res = run_bass_kernel_spmd(nc, in_maps, core_ids=list(range(8)),
                               trace=trace, tmpdir=tmpdir)
    kernel.last_exec_time_ns = res.exec_time_ns
    outs = [r["out"] for r in res.results]
    full = np.empty((2, S, D), dtype=np.float32)
    for b in range(2):
        full[b] = outs[4 * b].astype(np.float32)
        full[b] += outs[4 * b + 1].astype(np.float32)
        full[b] += outs[4 * b + 2].astype(np.float32)
        full[b] += outs[4 * b + 3].astype(np.float32)
    full += b_o[None, None, :]
    return full


# revision 28
# speedup vs baseline: 1.1756x; 1.1756x over previous
"""GQA causal attention (LLaMA rope) on 8 TRN2 NeuronCores.

Sharding: 8 cores = 2 batches x 4 kv-head-groups. Each core owns one batch
element, 2 kv heads and their 8 query heads (q head h uses kv head h % 8).
Host prep transposes/casts inputs (x^T bf16, pair-permuted w_qkv columns so
RoPE becomes contiguous-partition-block ops, slab-concatenated weight
layouts for single-DMA loads, tiled cos/sin tables, doubled causal triangle
mask, identity for PE transposes). On-device per core:
  1. qkv^T = w_all^T @ x^T via bf16 matmuls (transposed layout: head-dim on
     partitions) + RoPE on DVE + partition re-layout, interleaved per chunk
     so DVE stays ahead. v is also computed transposed (N=512 moving, 4x
     fewer PE instructions than the natural layout) then PE-transposed back
     per 128-block into the [v | ones] PV stationary tiles.
  2. Flash-style attention, S^T layout ([k-rows, q-cols]) over 512-wide q
     windows. The two heads of a pair share one kv head, so their score
     tiles live in one [128, 1024] PSUM pair tile: QK^T per head (K=32
     row-packed x1/x2 pair, PE-array row groups 0/64 run concurrently),
     ONE exp instruction per k-block covering both halves (3D AP), causal
     mask multiply on both diagonal blocks in one op, and ONE PV matmul
     per k-block accumulating both heads ([v | ones] stationary, row 64 =
     softmax sums). Output projection of the previous window drains into
     the current window's k-block slots to fill PE bubbles.
  3. Per window: sum rows gathered by ScalarE into one [8, W] tile, ONE
     DVE reciprocal, bf16 broadcast (GpSimd) and 4x-mode normalize
     multiplies, then out_partial = attn^T.T @ w_o_shard, written bf16.
Host gather: out[b] = sum of the 4 head-group partials + b_o.
"""

import os
import sys

sys.path.insert(0, "/opt/trn_rl_repo")

from contextlib import ExitStack

import numpy as np
import ml_dtypes

import concourse.bass as bass
import concourse.mybir as mybir
from concourse import bacc
import concourse.tile as tile
from concourse.bass import ds, ts

dt = mybir.dt
F32 = dt.float32
BF16 = dt.bfloat16
AF = mybir.ActivationFunctionType

D = 2048          # d_model
HD = 64           # head dim
NQ = 8            # local q heads per core
NKV = 2           # local kv heads per core
QR = NQ * HD      # 512 local q rows
ROPE_THETA = 10000.0
BF = ml_dtypes.bfloat16


PAIR_PV = os.environ.get("V4_PAIR_PV", "0") == "1"     # one 3D-AP PV matmul (fails walrus ISA check)
PAIR_EXP = os.environ.get("V4_PAIR_EXP", "1") == "1"   # one 3D-AP exp per kb
PAIR_MASK = os.environ.get("V4_PAIR_MASK", "1") == "1" # one 3D-AP mask per kb
VT_TRANS = os.environ.get("V4_VT_TRANS", "1") == "1"   # transposed v proj


def make_windows(S):
    ws = []
    qb = 0
    while qb < S:
        w = min(512, S - qb)
        ws.append((qb, w))
        qb += w
    return ws


def build_nc(S):
    """Build the single-core Bass/Tile graph for sequence length S."""
    SC = min(512, S)          # seq chunk for phase-1 matmuls
    NSLAB = D // 128          # 16 contraction slabs
    NCH = S // SC             # phase-1 chunks
    NST = S // 128            # seq 128-blocks

    nc = bacc.Bacc("TRN2", target_bir_lowering=False, debug=False)

    xt = nc.dram_tensor("xt", [D, S], BF16, kind="ExternalInput").ap()
    wall = nc.dram_tensor("wall", [128, NSLAB * 640], BF16, kind="ExternalInput").ap()
    wv = nc.dram_tensor("wv", [128, NSLAB * NKV * HD], BF16, kind="ExternalInput").ap()
    wo = nc.dram_tensor("wo", [128, 4 * D], BF16, kind="ExternalInput").ap()
    cs = nc.dram_tensor("cs", [128, S], BF16, kind="ExternalInput").ap()
    sn = nc.dram_tensor("sn", [128, S], BF16, kind="ExternalInput").ap()
    tri2 = nc.dram_tensor("tri2", [128, 256], BF16, kind="ExternalInput").ap()
    idm = nc.dram_tensor("idm", [128, 128], BF16, kind="ExternalInput").ap()
    out = nc.dram_tensor("out", [S, D], BF16, kind="ExternalOutput").ap()

    with tile.TileContext(nc) as tc, ExitStack() as ctx:
        const = ctx.enter_context(tc.tile_pool(name="const", bufs=1))
        tmp = ctx.enter_context(tc.tile_pool(name="tmp", bufs=2))
        ptp = ctx.enter_context(tc.tile_pool(name="ptp", bufs=3))
        recp = ctx.enter_context(tc.tile_pool(name="recp", bufs=2))
        mmp = ctx.enter_context(tc.tile_pool(name="mmp", bufs=2, space="PSUM"))
        pvp = ctx.enter_context(tc.tile_pool(name="pvp", bufs=2, space="PSUM"))

        # ---- persistent SBUF tensors -------------------------------------
        xt_sb = const.tile([128, NSLAB * S], BF16, tag="xt")
        wall_sb = const.tile([128, NSLAB * 640], BF16, tag="wall")
        wv_sb = const.tile([128, NSLAB * NKV * HD], BF16, tag="wv")
        cs_sb = const.tile([128, S], BF16, tag="cs")
        sn_sb = const.tile([128, S], BF16, tag="sn")
        tri2_sb = const.tile([128, 256], BF16, tag="tri2")
        id_sb = const.tile([128, 128], BF16, tag="idm")
        # post-rope q/k in transposed layout (bf16)
        qx1_sb = [const.tile([128, S], BF16, tag=f"qx1{i}", name=f"qx1{i}") for i in range(2)]
        qx2_sb = [const.tile([128, S], BF16, tag=f"qx2{i}", name=f"qx2{i}") for i in range(2)]
        kx_sb = const.tile([128, S], BF16, tag="kx")  # rows 0:64 x1, 64:128 x2
        # v^T staging + v natural [v_kv0 | 1 | v_kv1 | 1] PV stationaries
        vt_sb = const.tile([128, S], BF16, tag="vt")
        vones = [const.tile([128, 2 * (HD + 1)], BF16, tag=f"vo{i}", name=f"vo{i}")
                 for i in range(NST)]
        # attention output, transposed (head-dim rows): 4 tiles of 128 rows
        att_sb = [const.tile([128, S], BF16, tag=f"at{i}", name=f"at{i}") for i in range(4)]
        # partition-aligned attention inputs (own space so relayout can run
        # ahead of the xt readers): per head pair [x1(32)|x2(32)] strips
        qh_sb = [const.tile([128, S], BF16, tag=f"qh{i}", name=f"qh{i}")
                 for i in range(4)]
        kt2_sb = [const.tile([128, S], BF16, tag=f"kt2{i}", name=f"kt2{i}")
                  for i in range(NKV)]

        # ---- input DMAs: wall chunks interleaved with the xt slabs that
        # consume them so phase-1 starts within a few us
        def dma_in():
            for c in range(4):
                nc.sync.dma_start(wall_sb[:, ds(c * 4 * 640, 4 * 640)],
                                  wall[:, ds(c * 4 * 640, 4 * 640)])
                nc.sync.dma_start(
                    xt_sb[:, ds(4 * c * S, 4 * S)],
                    xt[ds(512 * c, 512), :].rearrange("(i p) s -> p i s",
                                                      p=128))
                if c == 0:
                    nc.sync.dma_start(cs_sb[:, :], cs[:, :])
                    nc.sync.dma_start(sn_sb[:, :], sn[:, :])
                elif c == 1:
                    nc.sync.dma_start(wv_sb[:, :], wv[:, :])
                elif c == 2:
                    nc.sync.dma_start(id_sb[:, :], idm[:, :])
                    nc.sync.dma_start(tri2_sb[:, :], tri2[:, :])
        dma_in()

        # ---- phase 1: qkv^T projection + RoPE + relayout -----------------
        def rope(x1, x2, d1, d2, c, s, P):
            """d1 = x1*c - x2*s ; d2 = x1*s + x2*c (writes bf16).

            PSUM f32 inputs are cast to bf16 once so the six elementwise ops
            run in the DVE 4x (2-byte packed SBUF) mode."""
            xb1 = tmp.tile([P, SC], BF16, tag="ra")
            xb2 = tmp.tile([P, SC], BF16, tag="rb")
            nc.vector.tensor_copy(xb1[:, :], x1)
            nc.vector.tensor_copy(xb2[:, :], x2)
            a = tmp.tile([P, SC], BF16, tag="rc")
            b = tmp.tile([P, SC], BF16, tag="rc")
            nc.vector.tensor_mul(a[:, :], xb1[:, :], c)
            nc.vector.tensor_mul(b[:, :], xb2[:, :], s)
            nc.vector.tensor_sub(d1, a[:, :], b[:, :])
            a2 = tmp.tile([P, SC], BF16, tag="rb")
            b2 = tmp.tile([P, SC], BF16, tag="ra")
            nc.vector.tensor_mul(a2[:, :], xb1[:, :], s)
            nc.vector.tensor_mul(b2[:, :], xb2[:, :], c)
            nc.vector.tensor_add(d2, a2[:, :], b2[:, :])

        def proj_chunk(sc_i):
            scs = ds(sc_i * SC, SC)
            for t in range(2):  # q tile pairs: x1 rows (t), x2 rows (2+t)
                ps1 = mmp.tile([128, SC], F32, tag="mm")
                for kk in range(NSLAB):
                    nc.tensor.matmul(ps1[:, :], wall_sb[:, ds(640 * kk + 128 * t, 128)],
                                     xt_sb[:, ds(S * kk + sc_i * SC, SC)],
                                     start=(kk == 0), stop=(kk == NSLAB - 1))
                ps2 = mmp.tile([128, SC], F32, tag="mm")
                for kk in range(NSLAB):
                    nc.tensor.matmul(ps2[:, :], wall_sb[:, ds(640 * kk + 256 + 128 * t, 128)],
                                     xt_sb[:, ds(S * kk + sc_i * SC, SC)],
                                     start=(kk == 0), stop=(kk == NSLAB - 1))
                rope(ps1[:, :], ps2[:, :],
                     qx1_sb[t][:, scs], qx2_sb[t][:, scs],
                     cs_sb[:, scs], sn_sb[:, scs], 128)
            # k tile: psum rows 0:64 = kx1, 64:128 = kx2
            psk = mmp.tile([128, SC], F32, tag="mm")
            for kk in range(NSLAB):
                nc.tensor.matmul(psk[:, :], wall_sb[:, ds(640 * kk + 512, 128)],
                                 xt_sb[:, ds(S * kk + sc_i * SC, SC)],
                                 start=(kk == 0), stop=(kk == NSLAB - 1))
            rope(psk[0:64, :], psk[64:128, :],
                 kx_sb[0:64, scs], kx_sb[64:128, scs],
                 cs_sb[0:64, scs], sn_sb[0:64, scs], 64)
            # v^T projection for this chunk (N=512 moving like q/k)
            if VT_TRANS:
                psv = mmp.tile([128, SC], F32, tag="mm")
                for kk in range(NSLAB):
                    nc.tensor.matmul(psv[:, :],
                                     wv_sb[:, ds(NKV * HD * kk, NKV * HD)],
                                     xt_sb[:, ds(S * kk + sc_i * SC, SC)],
                                     start=(kk == 0), stop=(kk == NSLAB - 1))
                nc.vector.tensor_copy(vt_sb[:, scs], psv[:, :])
            # partition-aligned re-layout for attention: interleave per head
            # [x1(32) | x2(32)] strips, two heads per tile; replicate each kv
            # head into both strips.
            for qt in range(4):
                for u in range(2):  # head 2*qt + u
                    l = 2 * qt + u
                    t, r0 = l // 4, 32 * (l % 4)
                    nc.vector.tensor_copy(qh_sb[qt][64 * u:64 * u + 32, scs],
                                          qx1_sb[t][r0:r0 + 32, scs])
                    nc.vector.tensor_copy(
                        qh_sb[qt][64 * u + 32:64 * u + 64, scs],
                        qx2_sb[t][r0:r0 + 32, scs])
            for j in range(NKV):
                for u in range(2):
                    nc.vector.tensor_copy(kt2_sb[j][64 * u:64 * u + 32, scs],
                                          kx_sb[32 * j:32 * j + 32, scs])
                    nc.vector.tensor_copy(
                        kt2_sb[j][64 * u + 32:64 * u + 64, scs],
                        kx_sb[64 + 32 * j:64 + 32 * j + 32, scs])

        def v_transpose(st_lo, st_hi):
            for st_i in range(st_lo, st_hi):
                if VT_TRANS:
                    tr = mmp.tile([128, 128], BF16, tag="mm", name=f"tr{st_i}")
                    nc.tensor.transpose(tr[:, :], vt_sb[:, ts(st_i, 128)],
                                        id_sb[:, :])
                else:
                    tr = mmp.tile([128, 128], F32, tag="mm", name=f"tr{st_i}")
                    for kk in range(NSLAB):
                        nc.tensor.matmul(tr[:, :],
                                         xt_sb[:, ds(S * kk + st_i * 128, 128)],
                                         wv_sb[:, ds(NKV * HD * kk, NKV * HD)],
                                         start=(kk == 0),
                                         stop=(kk == NSLAB - 1))
                vt = vones[st_i]
                nc.vector.tensor_copy(vt[:, 0:HD], tr[:, 0:HD])
                nc.vector.tensor_copy(vt[:, HD + 1:2 * HD + 1], tr[:, HD:2 * HD])
                nc.vector.memset(vt[:, HD:HD + 1], 1.0)
                nc.vector.memset(vt[:, 2 * HD + 1:2 * HD + 2], 1.0)

        half = (NCH + 1) // 2
        for sc_i in range(half):
            proj_chunk(sc_i)
        v_transpose(0, half * SC // 128)
        for sc_i in range(half, NCH):
            proj_chunk(sc_i)
        v_transpose(half * SC // 128, NST)

        # w_o loads reuse the xt slab slots (all xt reads are done by now)
        wo_sb = const.tile([128, 4 * D], BF16, tag="xt", name="wos")
        nc.sync.dma_start(wo_sb[:, :], wo[:, :])

        # ---- phases 2+3: attention + fused output projection -------------
        # Window-outer / head-pair-inner. PV lags QK/exp by one k-block so
        # the PE never waits on ScalarE. Output projection of window w-1 is
        # drained into window w's kb slots to fill PE bubbles while ACT
        # runs exp; remaining groups flush at the end.
        pending = []

        def queue_outproj(qb, W):
            for st_i in range(qb // 128, (qb + W) // 128):
                for nt in range(D // 512):
                    def g(st_i=st_i, nt=nt):
                        po = mmp.tile([128, 512], F32, tag="mm",
                                      name=f"po{st_i}_{nt}")
                        for kk in range(4):
                            nc.tensor.matmul(po[:, :],
                                             att_sb[kk][:, ts(st_i, 128)],
                                             wo_sb[:, ds(D * kk + 512 * nt, 512)],
                                             start=(kk == 0), stop=(kk == 3))
                        ot = tmp.tile([128, 512], BF16, tag="ot",
                                      name=f"ot{st_i}_{nt}")
                        nc.vector.tensor_copy(ot[:, :], po[:, :])
                        nc.sync.dma_start(
                            out[ds(st_i * 128, 128), ts(nt, 512)], ot[:, :])
                    pending.append(g)

        def pair3(t, off, ln, plo=0, phi=128):
            """[plo:phi, (2, ln)] AP over both halves of a [*, 2*W] tile."""
            return t[plo:phi, :].rearrange("p (u n) -> p u n", u=2)[
                :, :, ds(off, ln)]

        for wi, (qb, W) in enumerate(make_windows(S)):
            nkb = (qb + W) // 128
            last_w = qb + W >= S
            # hold 4 outproj groups for the window-end normalize chain
            budget = len(pending) - (0 if last_w else 4)
            for hp in range(4):
                kvl = hp // 2  # both heads of the pair share this kv head
                pvt = pvp.tile([128, 2 * W], F32, tag="pv",
                               name=f"pv{hp}_{qb}")
                lag = None  # (kb, pt, o)
                for kb in range(nkb):
                    kpos = kb * 128
                    o = max(kpos - qb, 0)
                    stp = mmp.tile([128, 2 * W], F32, tag="mm",
                                   name=f"st{hp}_{qb}_{kb}")
                    pt = ptp.tile([128, 2 * W], BF16, tag="pt",
                                  name=f"pt{hp}_{qb}_{kb}")
                    # QK^T: one K=64 matmul per head; pair lands in distinct
                    # row groups (base 0/64) -> concurrent on PE. Each half
                    # of the pair tile is one PSUM bank (W=512).
                    for u in range(2):
                        b0 = 64 * u
                        nc.tensor.matmul(
                            stp[:, ds(u * W + o, W - o)],
                            kt2_sb[kvl][b0:b0 + 64, ds(kpos, 128)],
                            qh_sb[hp][b0:b0 + 64, ds(qb + o, W - o)],
                            start=True, stop=True)
                    # exp(S/8) -> bf16 P^T, both heads in one instruction
                    if PAIR_EXP:
                        nc.scalar.activation(pair3(pt, o, W - o),
                                             pair3(stp, o, W - o),
                                             AF.Exp, scale=0.125)
                    else:
                        for u in range(2):
                            nc.scalar.activation(
                                pt[:, ds(u * W + o, W - o)],
                                stp[:, ds(u * W + o, W - o)],
                                AF.Exp, scale=0.125)
                    # causal triangle mask on both diagonal blocks
                    if kpos >= qb:
                        if PAIR_MASK:
                            nc.vector.tensor_mul(pair3(pt, o, 128),
                                                 pair3(pt, o, 128),
                                                 tri2_sb[:, :])
                        else:
                            for u in range(2):
                                nc.vector.tensor_mul(
                                    pt[:, ds(u * W + o, 128)],
                                    pt[:, ds(u * W + o, 128)],
                                    tri2_sb[:, 0:128])

                    def pv_mm(lkb, lpt, lo):
                        if PAIR_PV:
                            nc.tensor.matmul(
                                pair3(pvt, lo, W - lo, 0, 65),
                                vones[lkb][:, ds(kvl * (HD + 1), HD + 1)],
                                pair3(lpt, lo, W - lo),
                                start=(lkb == 0), stop=(lkb == nkb - 1))
                        else:
                            for u in range(2):
                                nc.tensor.matmul(
                                    pvt[0:65, ds(u * W + lo, W - lo)],
                                    vones[lkb][:, ds(kvl * (HD + 1), HD + 1)],
                                    lpt[:, ds(u * W + lo, W - lo)],
                                    start=(lkb == 0), stop=(lkb == nkb - 1))

                    if lag is not None:
                        pv_mm(*lag)
                    # outproj of window w-1 depends on its normalize chain
                    # (recip on DVE); drain only after ~2 head-pairs of PE
                    # work so a pop never blocks the in-order PE queue
                    if hp >= 2 and pending and budget > 0:
                        pending.pop(0)()
                        budget -= 1
                    lag = (kb, pt, o)
                pv_mm(*lag)
                # offload pv psum -> SBUF bf16 (frees the psum slot); pair
                # sum rows gathered into 32-aligned partition slots of the
                # shared per-window tile (engines require 32-aligned bases)
                if hp == 0:
                    sums8 = recp.tile([128, 2 * W], F32, tag="sums", bufs=1,
                                      name=f"sums{qb}")
                    pvsbs = []
                pvsb = recp.tile([HD, 2 * W], BF16, tag="pvsb", bufs=4,
                                 name=f"pvsb{hp}_{qb}")
                if hp % 2 == 0:
                    nc.vector.tensor_copy(pvsb[:, :], pvt[0:HD, :])
                else:
                    nc.scalar.activation(pvsb[:, :], pvt[0:HD, :], AF.Copy)
                nc.scalar.activation(sums8[32 * hp:32 * hp + 1, :],
                                     pvt[64:65, :], AF.Copy)
                pvsbs.append(pvsb)
            # ---- per-window normalize: one reciprocal for all 8 heads ----
            rec8 = recp.tile([128, 2 * W], F32, tag="rec8", bufs=1,
                             name=f"rec8{qb}")
            nc.vector.reciprocal(rec8[:, :], sums8[:, :])
            for hp in range(4):
                for u in range(2):
                    h = 2 * hp + u
                    # partition_broadcast needs a partition-0 zero-offset
                    # source: stage the head's rec row (f32 -> bf16 cast)
                    recs = recp.tile([1, W], BF16, tag="recs",
                                     name=f"recs{h}_{qb}")
                    nc.scalar.activation(
                        recs[:, :], rec8[32 * hp:32 * hp + 1, ds(u * W, W)],
                        AF.Copy)
                    bcs = recp.tile([HD, W], BF16, tag="bcs",
                                    name=f"bcs{h}_{qb}")
                    nc.gpsimd.partition_broadcast(bcs[:, :], recs[:, :],
                                                  channels=HD)
                    att_dst = att_sb[h // 2][64 * (h % 2):64 * (h % 2) + 64,
                                             ds(qb, W)]
                    nc.vector.tensor_mul(att_dst, pvsbs[hp][:, ds(u * W, W)],
                                         bcs[:, :])
                if pending:
                    pending.pop(0)()
            queue_outproj(qb, W)
        while pending:
            pending.pop(0)()
    nc.compile()
    return nc


# ---------------------------------------------------------------------------
# host-side prep / gather
# ---------------------------------------------------------------------------

def _slabcat(w, slab_rows=128):
    """[R, C] -> [slab_rows, (R//slab_rows)*C] slab-concatenated bf16."""
    r, c = w.shape
    n = r // slab_rows
    return np.ascontiguousarray(
        w.reshape(n, slab_rows, c).transpose(1, 0, 2).reshape(slab_rows, n * c)
    ).astype(BF)


def _core_inputs(x, w_qkv, w_o, S):
    """Per-core input dicts. Core 4*b+g: batch b, kv heads {2g, 2g+1}."""
    E = np.arange(0, HD, 2)
    O = np.arange(1, HD, 2)
    inv_freq = 1.0 / (ROPE_THETA ** (np.arange(0, HD, 2, dtype=np.float64) / HD))
    ang = np.arange(S, dtype=np.float64)[None, :] * inv_freq[:, None]  # [32,S]
    cs = np.tile(np.cos(ang), (4, 1)).astype(BF)
    sn = np.tile(np.sin(ang), (4, 1)).astype(BF)
    r = np.arange(128)
    tri = (r[:, None] <= r[None, :]).astype(BF)  # allow k<=q
    tri2 = np.ascontiguousarray(np.concatenate([tri, tri], axis=1))
    idm = np.eye(128, dtype=np.float32).astype(BF)

    maps = []
    for b in range(2):
        for g in range(4):
            qh = [2 * g, 2 * g + 8, 2 * g + 16, 2 * g + 24,
                  2 * g + 1, 2 * g + 9, 2 * g + 17, 2 * g + 25]
            kvh = [2 * g, 2 * g + 1]
            qx1_cols = np.concatenate([64 * h + E for h in qh])
            qx2_cols = np.concatenate([64 * h + O for h in qh])
            kx1_cols = np.concatenate([D + 64 * j + E for j in kvh])
            kx2_cols = np.concatenate([D + 64 * j + O for j in kvh])
            wall_cols = np.concatenate([qx1_cols, qx2_cols, kx1_cols, kx2_cols])
            wv_cols = np.concatenate(
                [D + NKV * 4 * HD + 64 * j + np.arange(HD) for j in kvh])
            wo_rows = np.concatenate([64 * h + np.arange(HD) for h in qh])
            maps.append({
                "xt": np.ascontiguousarray(x[b].T).astype(BF),
                "wall": _slabcat(w_qkv[:, wall_cols]),
                "wv": _slabcat(w_qkv[:, wv_cols]),
                "wo": _slabcat(w_o[wo_rows, :]),
                "cs": cs, "sn": sn, "tri2": tri2, "idm": idm,
            })
    return maps


def _install_axon_ntff_hook():
    """Provide antenv.axon_hooks via ctypes on libaxon_pjrt.so if missing."""
    try:
        from antenv.axon_hooks import get_axon_ntff_profile_hook  # noqa: F401
        return
    except ImportError:
        pass
    import contextlib
    import ctypes
    import types

    so_path = "/opt/axon/libaxon_pjrt.so"
    hook = None
    if os.path.exists(so_path):
        lib = ctypes.CDLL(so_path)
        if hasattr(lib, "axon_start_nrt_profile"):
            lib.axon_start_nrt_profile.argtypes = [
                ctypes.POINTER(ctypes.c_int64), ctypes.c_size_t]
            lib.axon_start_nrt_profile.restype = ctypes.c_int64
            lib.axon_stop_nrt_profile.argtypes = [ctypes.c_char_p]
            lib.axon_stop_nrt_profile.restype = ctypes.c_int64

            @contextlib.contextmanager
            def _hook(output_dir, device_ids):
                import jax
                jax.devices()
                if device_ids:
                    ids = (ctypes.c_int64 * len(device_ids))(*device_ids)
                    rc = lib.axon_start_nrt_profile(ids, len(device_ids))
                else:
                    rc = lib.axon_start_nrt_profile(None, 0)
                if rc != 0:
                    raise RuntimeError(f"axon_start_nrt_profile rc={rc}")
                try:
                    yield
                finally:
                    n = lib.axon_stop_nrt_profile(str(output_dir).encode())
                    print(f"ntff profile: {n} file(s) -> {output_dir}")

            hook = _hook

    import antenv
    mod = types.ModuleType("antenv.axon_hooks")
    state = {"hook": hook}
    mod.get_axon_ntff_profile_hook = lambda: state["hook"]
    mod.set_axon_ntff_profile_hook = lambda h: state.__setitem__("hook", h)
    sys.modules["antenv.axon_hooks"] = mod
    antenv.axon_hooks = mod


_NC_CACHE = {}


def kernel(x, w_qkv, b_qkv, w_o, b_o):
    x = np.asarray(x, dtype=np.float32)
    w_qkv = np.asarray(w_qkv, dtype=np.float32)
    w_o = np.asarray(w_o, dtype=np.float32)
    b_o = np.asarray(b_o, dtype=np.float32)
    S = x.shape[1]

    from concourse.bass_utils import run_bass_kernel_spmd

    if S not in _NC_CACHE:
        _NC_CACHE[S] = build_nc(S)
    nc = _NC_CACHE[S]

    in_maps = _core_inputs(x, w_qkv, w_o, S)
    trace = os.environ.get("BASS_KERNEL_TRACE", "0") == "1"
    tmpdir = None
    if trace:
        _install_axon_ntff_hook()
        import concourse.bass_utils as bu
        bu.upload_artifacts = lambda d: f"local://{d}"
        tmpdir = os.environ.get("BASS_KERNEL_TRACE_DIR") or None
        if tmpdir:
            import uuid
            tmpdir = os.path.join(tmpdir, uuid.uuid4().hex[:8])
            os.makedirs(tmpdir, exist_ok=True)
        kernel.last_trace_dir = tmpdir
    res = run_bass_kernel_spmd(nc, in_maps, core_ids=list(range(8)),
                               trace=trace, tmpdir=tmpdir)
    kernel.last_exec_time_ns = res.exec_time_ns
    outs = [r["out"] for r in res.results]
    full = np.empty((2, S, D), dtype=np.float32)
    for b in range(2):
        full[b] = outs[4 * b].astype(np.float32)
        full[b] += outs[4 * b + 1].astype(np.float32)
        full[b] += outs[4 * b + 2].astype(np.float32)
        full[b] += outs[4 * b + 3].astype(np.float32)
    full += b_o[None, None, :]
    return full
